# revision 17
# baseline (speedup 1.0000x reference)
"""MoE grouped-GEMM expert FFN (SwiGLU) for Trainium2, 8-core expert parallelism.

Contract: kernel(**inputs) takes FULL unsharded inputs, returns FULL output.

Strategy:
  - Host-side routing: tokens are contiguous per expert; split expert groups
    into chunks, band-assign chunks across 8 cores with an identical
    segment-capacity structure on every core (SPMD: one Bass program).
  - Per core, per segment: local GEMM1 (x @ w1w3) -> SwiGLU -> GEMM2 (h @ w2).
  - Host-side combine: scatter per-core output rows back to full output.

Matmul dtype fp16 (full PE rate, ~5e-4 rel err; fp8 DoubleRow was measured
at ~6e-2 end-to-end rel err with this data - over the 2e-2 gate - so fp16
is the fastest admissible dtype). PSUM/silu stay fp32; output stored fp16.

Layout choices:
  - All device inputs are host-repacked so every DMA loads long contiguous
    rows with few instructions (DMA issue costs ~0.6-1.3us per instruction
    on the sync sequencer; per-engine DMA bandwidth scales with run length).
  - x: packed per token tile as [tile, 128, 8*512] (hidden chunk k on the
    free dim) -> 1 DMA per token tile (issued on the scalar HWDGE queue so
    it doesn't contend with weight issues on sync).
  - w1w3: columns permuted so psum chunk c holds gate[64c:64c+64] on
    partitions 0:64 and up on 64:128 (SwiGLU = partition-slice op); rows
    packed as [S, 4, 128, 2*1408] (k-chunk pairs) -> 4 DMAs per segment.
  - w2: rows packed as [S, 128, 6*1024] (j on free dim; j=5 has 64 valid
    rows) -> 1 DMA per segment.
  - GEMM1 iterates k (contraction) outer / m inner within m-groups of <=4 so
    compute starts after ~0.5MB of DMA and segment boundaries pipeline.
  - GEMM2 uses w2 as stationary ([jw, 128 out-cols] slices) and h as moving
    -> every matmul streams the full token tile (no ceil(tt/128) token-chunk
    padding), output lands hidden-major in PSUM as [out-chunk 128, tokens].
  - Output stored fp16, transposed ([128, 8, cap_total] = p, oc, token);
    the host transposes back and upcasts (host time is not graded).
"""

import numpy as np

import concourse.bacc as bacc
import concourse.mybir as mybir
from concourse import tile
from concourse.bass_utils import run_bass_kernel_spmd

HIDDEN = 1024
INTER = 704
N_EXPERTS = 32
NCORES = 8
KC = HIDDEN // 128  # 8 k-chunks over hidden
MC = (2 * INTER) // 128  # 11 m-chunks over permuted gate|up dim
JC = (INTER + 127) // 128  # 6 j-chunks over inter for GEMM2 (last is 64 rows)
OC = HIDDEN // 128  # 8 output chunks over hidden for GEMM2
TT = 512  # token tile (moving free dim)
M_GROUPS = [(0, 2), (2, 4), (4, 6), (6, 8), (8, 10), (10, 11)]  # pair-sized m-groups
WARMUP_MM = 40

f32 = mybir.dt.float32
f16 = mybir.dt.float16

# Matmul input dtype. float16 runs ~1.7x faster than float32r at ~4.6e-4
# rel err (vs 2.5e-4 for f32r); PSUM accumulation stays fp32.
MM_DT = mybir.dt.float16
NP_DT = np.float16
ESZ = 2  # element size of MM_DT in bytes


def set_dtype(name):
    global MM_DT, NP_DT, ESZ
    if name == "f32r":
        MM_DT, NP_DT, ESZ = mybir.dt.float32r, np.float32, 4
    elif name == "f16":
        MM_DT, NP_DT, ESZ = mybir.dt.float16, np.float16, 2
    elif name == "bf16":
        MM_DT, NP_DT, ESZ = mybir.dt.bfloat16, np.float32, 2
    else:
        raise ValueError(name)


# Column permutation of w1w3's last dim (2*INTER): m-chunks come in
# (gate, up) pairs of full 128-row blocks so SwiGLU runs full-width
# [128, tt] ACT/DVE ops. chunk 2j = gate[128j:128j+128], chunk 2j+1 =
# up[128j:128j+128] for j<5; the last chunk holds the 64-row tails.
_PERM = np.empty(2 * INTER, dtype=np.int64)
for _j in range(5):
    _PERM[256 * _j : 256 * _j + 128] = np.arange(128 * _j, 128 * _j + 128)
    _PERM[256 * _j + 128 : 256 * _j + 256] = INTER + np.arange(
        128 * _j, 128 * _j + 128
    )
_PERM[1280:1344] = np.arange(640, 704)
_PERM[1344:1408] = INTER + np.arange(640, 704)


def _to_np_dt(a):
    if MM_DT == mybir.dt.bfloat16:
        b = np.asarray(a, dtype=np.float32).copy()
        v = b.view(np.uint32)
        v += 0x8000
        v &= 0xFFFF0000
        return b
    return np.asarray(a, dtype=NP_DT)


def _ceil16(x):
    return -(-int(x) // 16) * 16


def _make_chunks(counts, starts, tmax):
    chunks = []  # (n, expert, tok_start)
    for e in range(N_EXPERTS):
        n = int(counts[e])
        a = int(starts[e])
        if n <= 0:
            continue
        nparts = -(-n // tmax)
        base, rem = divmod(n, nparts)
        off = 0
        for p in range(nparts):
            ln = base + (1 if p < rem else 0)
            if ln > 0:
                chunks.append((ln, e, a + off))
                off += ln
    return chunks


def _caps_of(sizes):
    """Band caps for a sorted-desc multiset of piece sizes."""
    s = sorted(sizes, reverse=True)
    return [max(16, _ceil16(s[i])) for i in range(0, len(s), NCORES)]


def _pe_time(caps, lw):
    """Per-core PE time model (ns) for the v2 structure."""
    t = 0.0
    for C in caps:
        for t0 in range(0, C, TT):
            tt = min(TT, C - t0)
            mm = tt * 0.4267 + 8
            t += 88 * max(lw, mm) + 48 * max(lw, mm)
    return t


def _plan(counts):
    """Balance (expert, token-chunk) pieces across NCORES cores.

    Chunks are sorted by size and dealt in bands of 8 (one per core): slot s
    capacity = the largest chunk in band s, which minimizes total capacity
    for a given chunk multiset. The split threshold trades segment count
    (weight DMA traffic) against padding (PE + activation traffic). A
    hill-climb then moves 16-token quanta between sibling pieces of the
    same expert to shave band maxima.
    """
    starts = np.zeros(N_EXPERTS, dtype=np.int64)
    np.cumsum(counts[:-1], out=starts[1:])

    lw = 210.0 if ESZ == 4 else 100.0  # per-MM floor (ldweights-bound), ns
    w_seg = (HIDDEN * 2 * INTER + JC * 128 * HIDDEN) * ESZ

    def score_of(chunks):
        caps = _caps_of([c[0] for c in chunks]) if chunks else [16]
        S = len(caps)
        cap_total = sum(caps)
        dma_t = (S * w_seg + cap_total * HIDDEN * (ESZ + 2)) / 410.0  # bytes/ns
        pe_t = _pe_time(caps, lw)
        return max(dma_t, pe_t) + 0.2 * min(dma_t, pe_t), caps

    best = None
    for tmax in (4096, 2048, 1536, 1024, *range(256, 1025, 16)):
        chunks = _make_chunks(counts, starts, max(1, tmax))
        if not chunks:
            chunks = [(0, None, 0)]
        score, caps = score_of(chunks)
        if best is None or score < best[0]:
            best = (score, chunks, caps)

    score, chunks, caps = best

    # Hill-climb: move 16-token quanta between pieces of the same expert.
    chunks = [list(c) for c in chunks]
    by_e = {}
    for i, (n, e, a) in enumerate(chunks):
        by_e.setdefault(e, []).append(i)
    rng = np.random.default_rng(0)
    cur = score
    for _ in range(600):
        es = [e for e, idxs in by_e.items() if len(idxs) > 1 and e is not None]
        if not es:
            break
        e = es[int(rng.integers(0, len(es)))]
        idxs = by_e[e]
        i, j = rng.integers(0, len(idxs), 2)
        i, j = idxs[int(i)], idxs[int(j)]
        if i == j:
            continue
        q = 16
        if chunks[i][0] <= q:
            continue
        chunks[i][0] -= q
        chunks[j][0] += q
        s2, _ = score_of([tuple(c) for c in chunks])
        if s2 <= cur:
            cur = s2
        else:
            chunks[i][0] += q
            chunks[j][0] -= q

    # Rebuild contiguous starts per expert (piece order within expert).
    by_e2 = {}
    for c in chunks:
        by_e2.setdefault(c[1], []).append(c)
    for e, cs in by_e2.items():
        if e is None:
            continue
        off = int(starts[e])
        for c in cs:
            c[2] = off
            off += c[0]

    chunks = [tuple(c) for c in chunks if c[0] > 0]
    if not chunks:
        chunks = [(0, None, 0)]
    chunks.sort(key=lambda c: -c[0])
    S = -(-len(chunks) // NCORES)
    caps = []
    for s in range(S):
        band = chunks[NCORES * s : NCORES * (s + 1)]
        caps.append(max(16, _ceil16(band[0][0])))
    offs = np.concatenate([[0], np.cumsum(caps)[:-1]]).astype(np.int64)
    cap_total = int(sum(caps))

    assign = [[] for _ in range(NCORES)]
    for s in range(S):
        band = chunks[NCORES * s : NCORES * (s + 1)]
        for c in range(NCORES):
            if c < len(band):
                n, e, a = band[c]
                assign[c].append((e, a, n))
            else:
                assign[c].append((None, 0, 0))
    return assign, caps, offs, cap_total


def _tiles_of(caps):
    out = []
    for s, C in enumerate(caps):
        for t0 in range(0, C, TT):
            out.append((s, t0, min(TT, C - t0)))
    return out


def _build(S, caps, cap_total):
    """Build the SPMD Bass program for one core's segment structure."""
    nc = bacc.Bacc(
        "TRN2",
        target_bir_lowering=False,
        debug=False,
        enable_asserts=False,
        num_devices=NCORES,
    )

    tiles = _tiles_of(caps)
    NT = len(tiles)
    offs = np.concatenate([[0], np.cumsum(caps)[:-1]]).astype(np.int64)

    xt_d = nc.declare_dram_parameter("xt", [NT, 128, KC * TT], MM_DT, isOutput=False)
    w13_d = nc.declare_dram_parameter(
        "w13", [S, 4, 128, 2 * 2 * INTER], MM_DT, isOutput=False
    )
    w2_d = nc.declare_dram_parameter(
        "w2", [S, 128, JC * HIDDEN], MM_DT, isOutput=False
    )
    # Output layout: [128, OC, cap_total] fp16: out_d[p, oc, g] holds
    # out[token g][128*oc + p]. The oc-major layout keeps the PSUM->SBUF
    # casts contiguous (a token-interleaved layout made the DVE cast 4x
    # slower: strided 16B writes); the 3D store has tt*2-byte rows, which
    # is acceptable on the SWDGE trickle path.
    out_d = nc.declare_dram_parameter(
        "out", [128, OC, cap_total], f16, isOutput=True
    )

    big = ESZ == 4
    w13_bufs = 6 if big else 12
    w2_bufs = 2 if big else 3
    xt_bufs = 3 if big else 4

    with tile.TileContext(nc) as tc:
        with (
            tc.tile_pool(name="w13p", bufs=w13_bufs) as w13p,
            tc.tile_pool(name="w2p", bufs=w2_bufs) as w2p,
            tc.tile_pool(name="xtp", bufs=xt_bufs) as xtp,
            tc.tile_pool(name="hp", bufs=12) as hp,
            tc.tile_pool(name="sgp", bufs=6) as sgp,
            tc.tile_pool(name="outp", bufs=3) as outp,
            tc.tile_pool(name="ps1", bufs=4, space="PSUM") as ps1,
            tc.tile_pool(name="ps2", bufs=4, space="PSUM") as ps2,
        ):
            # All loads ride the sync HWDGE queue in issue order. (Spreading
            # loads across the sync+scalar HWDGE queues was measured to
            # HALVE both queues' per-engine rates - the 16 DMA engines are
            # shared - and the semaphore-pool recycling serialized issues;
            # first real matmul slipped from 12.6us to 18.6us.)
            # HAM warmup: PE sits at 1.2GHz until ~4us of sustained matmul
            # activity; run throwaway matmuls while the first DMAs fly.
            warm_sb = sgp.tile([128, 128], MM_DT, tag="warm", name="warm_sb")
            nc.vector.memset(warm_sb[:], 0.0)
            warm_ps = ps1.tile([128, 128], f32, tag="pg", name="warm_ps",
                               padded_shape=[128, TT])
            for _w in range(WARMUP_MM):
                nc.tensor.matmul(
                    warm_ps[:, 0:128],
                    warm_sb[:, 0:128],
                    warm_sb[:, 0:128],
                    start=True,
                    stop=True,
                )

            tix = 0
            for s in range(S):
                C = caps[s]
                off = int(offs[s])

                # First token tile's xt ahead of the weights: the queues
                # drain roughly in issue order and the first matmul of the
                # segment needs (xt, w13 pair 0). For segment 0 the first
                # chunks are split into separate tiles so the first real
                # matmul gates on ~0.5MB instead of 1.4MB (tile-framework
                # deps are tile-granular); the k-outer loop then consumes
                # k-chunks as they land.
                tt0 = min(TT, C)
                if s == 0:
                    xt_a = xtp.tile([128, 2 * tt0], MM_DT, tag="xta",
                                    name="xta", padded_shape=[128, 2 * TT])
                    nc.sync.dma_start(out=xt_a[:], in_=xt_d[0, :, 0 : 2 * tt0])
                    w13k = []
                    for k in range(2):
                        w13kt = w13p.tile([128, 2 * INTER], MM_DT, tag="w13t",
                                          name=f"w13k{k}",
                                          padded_shape=[128, 2 * 2 * INTER])
                        nc.sync.dma_start(
                            out=w13kt[:],
                            in_=w13_d[0, 0, :, k * 2 * INTER : (k + 1) * 2 * INTER],
                        )
                        w13k.append(w13kt)
                    xt_b = xtp.tile([128, (KC - 2) * tt0], MM_DT, tag="xtt",
                                    name="xtb", padded_shape=[128, KC * TT])
                    nc.sync.dma_start(out=xt_b[:], in_=xt_d[0, :, 2 * tt0 : KC * tt0])
                    xt_first = None
                else:
                    xt_first = xtp.tile([128, KC * tt0], MM_DT, tag="xtt",
                                        name=f"xtt{tix}",
                                        padded_shape=[128, KC * TT])
                    nc.sync.dma_start(
                        out=xt_first[:], in_=xt_d[tix, :, 0 : KC * tt0]
                    )

                # Weights for this segment, in first-use (k) order.
                w13_t = []
                for kp in range(4):
                    if s == 0 and kp == 0:
                        w13_t.append(None)  # covered by w13k split tiles
                        continue
                    w13t = w13p.tile([128, 2 * 2 * INTER], MM_DT, tag="w13t",
                                     name=f"w13t{s}_{kp}")
                    nc.sync.dma_start(out=w13t[:], in_=w13_d[s, kp])
                    w13_t.append(w13t)
                w2t = w2p.tile([128, JC * HIDDEN], MM_DT, tag="w2t", name=f"w2t{s}")
                nc.sync.dma_start(out=w2t[:], in_=w2_d[s])

                def w13_ap(k, m):
                    if s == 0 and k < 2:
                        return w13k[k][:, 128 * m : 128 * m + 128]
                    base = (k % 2) * 2 * INTER + 128 * m
                    return w13_t[k // 2][:, base : base + 128]

                def w2_ap(j, oc):
                    jw = min(128, INTER - 128 * j)
                    base = j * HIDDEN + 128 * oc
                    return w2t[0:jw, base : base + 128]

                for t0 in range(0, C, TT):
                    tt = min(TT, C - t0)
                    if t0 == 0 and s == 0:
                        xt_tile = None
                    elif t0 == 0:
                        xt_tile = xt_first
                    else:
                        xt_tile = xtp.tile([128, KC * tt], MM_DT, tag="xtt",
                                           name=f"xtt{tix}",
                                           padded_shape=[128, KC * TT])
                        nc.sync.dma_start(
                            out=xt_tile[:], in_=xt_d[tix, :, 0 : KC * tt]
                        )

                    def xt_ap(k):
                        if xt_tile is None:
                            if k < 2:
                                return xt_a[:, k * tt : (k + 1) * tt]
                            return xt_b[:, (k - 2) * tt : (k - 1) * tt]
                        return xt_tile[:, k * tt : (k + 1) * tt]

                    h_t = []
                    for j in range(JC):
                        jw = min(128, INTER - 128 * j)
                        ht = hp.tile([jw, tt], MM_DT, tag="ht", name=f"ht{tix}_{j}",
                                     padded_shape=[128, TT])
                        h_t.append(ht)

                    # GEMM1: k-outer within m-groups of 2 psum tiles.
                    for m_lo, m_hi in M_GROUPS:
                        pgs = {}
                        for m in range(m_lo, m_hi):
                            pgs[m] = ps1.tile([128, tt], f32, tag="pg",
                                              name=f"pg{m}",
                                              padded_shape=[128, TT])
                        for k in range(KC):
                            for m in range(m_lo, m_hi):
                                nc.tensor.matmul(
                                    pgs[m][:],
                                    w13_ap(k, m),
                                    xt_ap(k),
                                    start=(k == 0),
                                    stop=(k == KC - 1),
                                )
                        for m in range(m_lo, m_hi):
                            if m == MC - 1:
                                # tail chunk: [gate 64 | up 64] on partitions
                                sg = sgp.tile([64, tt], f32, tag="sg",
                                              name=f"sg{m}",
                                              padded_shape=[128, TT])
                                nc.scalar.activation(
                                    sg[:], pgs[m][0:64, :],
                                    mybir.ActivationFunctionType.Silu,
                                )
                                nc.vector.tensor_mul(
                                    h_t[JC - 1][0:64, :], sg[:],
                                    pgs[m][64:128, :],
                                )
                            elif m % 2 == 1:
                                sg = sgp.tile([128, tt], f32, tag="sg",
                                              name=f"sg{m}",
                                              padded_shape=[128, TT])
                                nc.scalar.activation(
                                    sg[:], pgs[m - 1][:],
                                    mybir.ActivationFunctionType.Silu,
                                )
                                nc.vector.tensor_mul(
                                    h_t[m // 2][:], sg[:], pgs[m][:]
                                )

                    # GEMM2: w2 stationary, h moving; output [oc*128, tokens]
                    # in PSUM, cast (contiguously) to fp16 into one out tile
                    # and stored with a 3D-AP DMA per token tile.
                    ob = outp.tile([128, OC * tt], f16, tag="ob", name=f"ob{tix}",
                                   padded_shape=[128, OC * TT])
                    for half in range(2):
                        pos = {}
                        for oc in range(4 * half, 4 * half + 4):
                            pos[oc] = ps2.tile([128, tt], f32, tag="po",
                                               name=f"po{oc}",
                                               padded_shape=[128, TT])
                        for j in range(JC):
                            for oc in range(4 * half, 4 * half + 4):
                                nc.tensor.matmul(
                                    pos[oc][:],
                                    w2_ap(j, oc),
                                    h_t[j][:],
                                    start=(j == 0),
                                    stop=(j == JC - 1),
                                )
                        for oc in range(4 * half, 4 * half + 4):
                            nc.vector.tensor_copy(
                                ob[:, oc * tt : (oc + 1) * tt], pos[oc][:]
                            )

                    # Stores ride the gpsimd (SWDGE) queue so they never
                    # block later loads on the sync sequencer; the last
                    # segment's store takes the HWDGE path. (Do NOT touch
                    # the scalar HWDGE queue: merely allocating the second
                    # HWDGE ring slowed the head's load stream by ~3us.)
                    src = ob[:].rearrange("p (o t) -> p o t", o=OC)
                    store_eng = nc.sync if s == S - 1 else nc.gpsimd
                    store_eng.dma_start(
                        out=out_d[:, :, off + t0 : off + t0 + tt],
                        in_=src,
                    )
                    tix += 1

    nc.compile()
    return nc


_BUILD_CACHE = {}


def _get_program(S, caps, cap_total):
    key = (S, tuple(caps), str(MM_DT))
    if key not in _BUILD_CACHE:
        _BUILD_CACHE[key] = _build(S, caps, cap_total)
    return _BUILD_CACHE[key]


def _pack_inputs(x, assign, caps, offs, cap_total, w13_perm):
    """Build per-core input dicts matching the device layouts."""
    tiles = _tiles_of(caps)
    NT = len(tiles)
    S = len(caps)
    in_maps = []
    for c in range(NCORES):
        xt_c = np.zeros((HIDDEN, cap_total), dtype=NP_DT)
        w13_c = np.zeros((S, 4, 128, 2 * 2 * INTER), dtype=NP_DT)
        w2_c = np.zeros((S, 128, JC * HIDDEN), dtype=NP_DT)
        for s, (e, a, n) in enumerate(assign[c]):
            if e is None or n <= 0:
                continue
            o = int(offs[s])
            xt_c[:, o : o + n] = _to_np_dt(x[a : a + n, :]).T
            w13_c[s] = (
                w13_perm["w13"][e]
                .reshape(4, 2, 128, 2 * INTER)
                .transpose(0, 2, 1, 3)
                .reshape(4, 128, 2 * 2 * INTER)
            )
            w2_c[s] = w13_perm["w2"][e]
        xt_pack = np.zeros((NT, 128, KC * TT), dtype=NP_DT)
        for tix, (s, t0, tt) in enumerate(tiles):
            o = int(offs[s])
            blk = xt_c[:, o + t0 : o + t0 + tt]
            xt_pack[tix, :, 0 : KC * tt] = (
                blk.reshape(KC, 128, tt).transpose(1, 0, 2).reshape(128, KC * tt)
            )
        in_maps.append({"xt": xt_pack, "w13": w13_c, "w2": w2_c})
    return in_maps


def _prep_weights(w1w3, w2):
    w13_perm = _to_np_dt(w1w3[:, :, _PERM])
    w2p_all = np.zeros((N_EXPERTS, JC * 128, HIDDEN), dtype=NP_DT)
    w2p_all[:, :INTER] = _to_np_dt(w2)
    w2_pack = (
        w2p_all.reshape(N_EXPERTS, JC, 128, HIDDEN)
        .transpose(0, 2, 1, 3)
        .reshape(N_EXPERTS, 128, JC * HIDDEN)
    )
    return {"w13": w13_perm, "w2": w2_pack}


def _run(x, tokens_per_expert, w1w3, w2, trace=False):
    x = np.ascontiguousarray(np.asarray(x, dtype=np.float32))
    counts = np.asarray(tokens_per_expert, dtype=np.int64).copy()
    w1w3 = np.asarray(w1w3, dtype=np.float32)
    w2 = np.asarray(w2, dtype=np.float32)

    T = x.shape[0]
    # Clip group sizes like ragged_dot: groups are consecutive; anything
    # beyond T is out of range.
    counts = np.maximum(counts, 0)
    cum = np.cumsum(counts)
    over = cum > T
    if over.any():
        first = int(np.argmax(over))
        prev = int(cum[first - 1]) if first > 0 else 0
        counts[first] = T - prev
        counts[first + 1 :] = 0

    assign, caps, offs, cap_total = _plan(counts)
    S = len(caps)
    nc = _get_program(S, caps, cap_total)

    packed_w = _prep_weights(w1w3, w2)
    in_maps = _pack_inputs(x, assign, caps, offs, cap_total, packed_w)

    extra = {}
    if trace:
        import os

        os.makedirs("/tmp/moe_prof", exist_ok=True)
        for f in os.listdir("/tmp/moe_prof"):
            os.unlink(os.path.join("/tmp/moe_prof", f))
        extra["tmpdir"] = "/tmp/moe_prof"
    res = run_bass_kernel_spmd(nc, in_maps, list(range(NCORES)), trace=trace, **extra)

    out_full = np.zeros((T, HIDDEN), dtype=np.float32)
    for c in range(NCORES):
        oc = res.results[c]["out"]  # [128, OC, cap_total] fp16
        arr = oc.transpose(1, 0, 2).reshape(HIDDEN, cap_total)
        for s, (e, a, n) in enumerate(assign[c]):
            if e is None or n <= 0:
                continue
            o = int(offs[s])
            out_full[a : a + n, :] = arr[:, o : o + n].T.astype(np.float32)
    return out_full, res


def kernel(x, tokens_per_expert, w1w3, w2, decoding=False, **_ignored):
    out, _ = _run(x, tokens_per_expert, w1w3, w2, trace=False)
    return out


# revision 21
# speedup vs baseline: 1.2059x; 1.2059x over previous
"""MoE grouped-GEMM expert FFN (SwiGLU) for Trainium2, 8-core expert parallelism.

Contract: kernel(**inputs) takes FULL unsharded inputs, returns FULL output.

Strategy:
  - Host-side routing: tokens are contiguous per expert; split expert groups
    into chunks, band-assign chunks across 8 cores with an identical
    segment-capacity structure on every core (SPMD: one Bass program).
  - Per core, per segment: local GEMM1 (x @ w1w3) -> SwiGLU -> GEMM2 (h @ w2).
  - Host-side combine: scatter per-core output rows back to full output.

Matmul dtype fp16 (full PE rate, ~5e-4 rel err; fp8 DoubleRow was measured
at ~6e-2 end-to-end rel err with this data - over the 2e-2 gate - so fp16
is the fastest admissible dtype). PSUM/silu stay fp32; output stored fp16.

Layout choices:
  - All device inputs are host-repacked so every DMA loads long contiguous
    rows with few instructions (DMA issue costs ~0.6-1.3us per instruction
    on the sync sequencer; per-engine DMA bandwidth scales with run length).
  - x: packed per token tile as [tile, 128, 8*512] (hidden chunk k on the
    free dim) -> 1 DMA per token tile (issued on the scalar HWDGE queue so
    it doesn't contend with weight issues on sync).
  - w1w3: columns permuted so psum chunk c holds gate[64c:64c+64] on
    partitions 0:64 and up on 64:128 (SwiGLU = partition-slice op); rows
    packed as [S, 4, 128, 2*1408] (k-chunk pairs) -> 4 DMAs per segment.
  - w2: rows packed as [S, 128, 6*1024] (j on free dim; j=5 has 64 valid
    rows) -> 1 DMA per segment.
  - GEMM1 iterates k (contraction) outer / m inner within m-groups of <=4 so
    compute starts after ~0.5MB of DMA and segment boundaries pipeline.
  - GEMM2 uses w2 as stationary ([jw, 128 out-cols] slices) and h as moving
    -> every matmul streams the full token tile (no ceil(tt/128) token-chunk
    padding), output lands hidden-major in PSUM as [out-chunk 128, tokens].
  - Output stored fp16, transposed ([128, 8, cap_total] = p, oc, token);
    the host transposes back and upcasts (host time is not graded).
"""

import numpy as np

import concourse.bacc as bacc
import concourse.mybir as mybir
from concourse import tile
from concourse.bass_utils import run_bass_kernel_spmd

HIDDEN = 1024
INTER = 704
N_EXPERTS = 32
NCORES = 8
KC = HIDDEN // 128  # 8 k-chunks over hidden
MC = (2 * INTER) // 128  # 11 m-chunks over permuted gate|up dim
JC = (INTER + 127) // 128  # 6 j-chunks over inter for GEMM2 (last is 64 rows)
OC = HIDDEN // 128  # 8 output chunks over hidden for GEMM2
TT = 512  # token tile (moving free dim)
M_GROUPS = [(0, 2), (2, 4), (4, 6), (6, 8), (8, 10), (10, 11)]  # pair-sized m-groups
WARMUP_SHORT = 38  # [128,128] matmuls at cold clock: ~4.1us, flips HAM to 8/8
WARMUP_LONG = 22  # [128,512] matmuls at full clock: ~213ns each, ~4.7us more

f32 = mybir.dt.float32
f16 = mybir.dt.float16

# Matmul input dtype. float16 runs ~1.7x faster than float32r at ~4.6e-4
# rel err (vs 2.5e-4 for f32r); PSUM accumulation stays fp32.
MM_DT = mybir.dt.float16
NP_DT = np.float16
ESZ = 2  # element size of MM_DT in bytes


def set_dtype(name):
    global MM_DT, NP_DT, ESZ
    if name == "f32r":
        MM_DT, NP_DT, ESZ = mybir.dt.float32r, np.float32, 4
    elif name == "f16":
        MM_DT, NP_DT, ESZ = mybir.dt.float16, np.float16, 2
    elif name == "bf16":
        MM_DT, NP_DT, ESZ = mybir.dt.bfloat16, np.float32, 2
    else:
        raise ValueError(name)


# Column permutation of w1w3's last dim (2*INTER): m-chunks come in
# (gate, up) pairs of full 128-row blocks so SwiGLU runs full-width
# [128, tt] ACT/DVE ops. chunk 2j = gate[128j:128j+128], chunk 2j+1 =
# up[128j:128j+128] for j<5; the last chunk holds the 64-row tails.
_PERM = np.empty(2 * INTER, dtype=np.int64)
for _j in range(5):
    _PERM[256 * _j : 256 * _j + 128] = np.arange(128 * _j, 128 * _j + 128)
    _PERM[256 * _j + 128 : 256 * _j + 256] = INTER + np.arange(
        128 * _j, 128 * _j + 128
    )
_PERM[1280:1344] = np.arange(640, 704)
_PERM[1344:1408] = INTER + np.arange(640, 704)


def _to_np_dt(a):
    if MM_DT == mybir.dt.bfloat16:
        b = np.asarray(a, dtype=np.float32).copy()
        v = b.view(np.uint32)
        v += 0x8000
        v &= 0xFFFF0000
        return b
    return np.asarray(a, dtype=NP_DT)


def _ceil16(x):
    return -(-int(x) // 16) * 16


def _make_chunks(counts, starts, tmax):
    chunks = []  # (n, expert, tok_start)
    for e in range(N_EXPERTS):
        n = int(counts[e])
        a = int(starts[e])
        if n <= 0:
            continue
        nparts = -(-n // tmax)
        base, rem = divmod(n, nparts)
        off = 0
        for p in range(nparts):
            ln = base + (1 if p < rem else 0)
            if ln > 0:
                chunks.append((ln, e, a + off))
                off += ln
    return chunks


def _caps_of(sizes):
    """Band caps for a sorted-desc multiset of piece sizes."""
    s = sorted(sizes, reverse=True)
    return [max(16, _ceil16(s[i])) for i in range(0, len(s), NCORES)]


def _pe_time(caps, lw):
    """Per-core PE time model (ns) for the v2 structure."""
    t = 0.0
    for C in caps:
        for t0 in range(0, C, TT):
            tt = min(TT, C - t0)
            mm = tt * 0.4267 + 8
            t += 88 * max(lw, mm) + 48 * max(lw, mm)
    return t


def _plan(counts):
    """Balance (expert, token-chunk) pieces across NCORES cores.

    Chunks are sorted by size and dealt in bands of 8 (one per core): slot s
    capacity = the largest chunk in band s, which minimizes total capacity
    for a given chunk multiset. The split threshold trades segment count
    (weight DMA traffic) against padding (PE + activation traffic). A
    hill-climb then moves 16-token quanta between sibling pieces of the
    same expert to shave band maxima.
    """
    starts = np.zeros(N_EXPERTS, dtype=np.int64)
    np.cumsum(counts[:-1], out=starts[1:])

    lw = 210.0 if ESZ == 4 else 100.0  # per-MM floor (ldweights-bound), ns
    w_seg = (HIDDEN * 2 * INTER + JC * 128 * HIDDEN) * ESZ

    def score_of(chunks):
        caps = _caps_of([c[0] for c in chunks]) if chunks else [16]
        S = len(caps)
        cap_total = sum(caps)
        dma_t = (S * w_seg + cap_total * HIDDEN * (ESZ + 2)) / 410.0  # bytes/ns
        pe_t = _pe_time(caps, lw)
        return max(dma_t, pe_t) + 0.2 * min(dma_t, pe_t), caps

    best = None
    for tmax in (4096, 2048, 1536, 1024, *range(256, 1025, 16)):
        chunks = _make_chunks(counts, starts, max(1, tmax))
        if not chunks:
            chunks = [(0, None, 0)]
        score, caps = score_of(chunks)
        if best is None or score < best[0]:
            best = (score, chunks, caps)

    score, chunks, caps = best

    # Hill-climb: move 16-token quanta between pieces of the same expert.
    chunks = [list(c) for c in chunks]
    by_e = {}
    for i, (n, e, a) in enumerate(chunks):
        by_e.setdefault(e, []).append(i)
    rng = np.random.default_rng(0)
    cur = score
    for _ in range(600):
        es = [e for e, idxs in by_e.items() if len(idxs) > 1 and e is not None]
        if not es:
            break
        e = es[int(rng.integers(0, len(es)))]
        idxs = by_e[e]
        i, j = rng.integers(0, len(idxs), 2)
        i, j = idxs[int(i)], idxs[int(j)]
        if i == j:
            continue
        q = 16
        if chunks[i][0] <= q:
            continue
        chunks[i][0] -= q
        chunks[j][0] += q
        s2, _ = score_of([tuple(c) for c in chunks])
        if s2 <= cur:
            cur = s2
        else:
            chunks[i][0] += q
            chunks[j][0] -= q

    # Rebuild contiguous starts per expert (piece order within expert).
    by_e2 = {}
    for c in chunks:
        by_e2.setdefault(c[1], []).append(c)
    for e, cs in by_e2.items():
        if e is None:
            continue
        off = int(starts[e])
        for c in cs:
            c[2] = off
            off += c[0]

    chunks = [tuple(c) for c in chunks if c[0] > 0]
    if not chunks:
        chunks = [(0, None, 0)]
    chunks.sort(key=lambda c: -c[0])
    S = -(-len(chunks) // NCORES)
    caps = []
    for s in range(S):
        band = chunks[NCORES * s : NCORES * (s + 1)]
        caps.append(max(16, _ceil16(band[0][0])))
    offs = np.concatenate([[0], np.cumsum(caps)[:-1]]).astype(np.int64)
    cap_total = int(sum(caps))

    assign = [[] for _ in range(NCORES)]
    for s in range(S):
        band = chunks[NCORES * s : NCORES * (s + 1)]
        for c in range(NCORES):
            if c < len(band):
                n, e, a = band[c]
                assign[c].append((e, a, n))
            else:
                assign[c].append((None, 0, 0))
    return assign, caps, offs, cap_total


def _tiles_of(caps):
    out = []
    for s, C in enumerate(caps):
        for t0 in range(0, C, TT):
            out.append((s, t0, min(TT, C - t0)))
    return out


def _build(S, caps, cap_total):
    """Build the SPMD Bass program for one core's segment structure."""
    nc = bacc.Bacc(
        "TRN2",
        target_bir_lowering=False,
        debug=False,
        enable_asserts=False,
        num_devices=NCORES,
    )

    tiles = _tiles_of(caps)
    NT = len(tiles)
    offs = np.concatenate([[0], np.cumsum(caps)[:-1]]).astype(np.int64)

    xt_d = nc.declare_dram_parameter("xt", [NT, 128, KC * TT], MM_DT, isOutput=False)
    w13_d = nc.declare_dram_parameter(
        "w13", [S, 4, 128, 2 * 2 * INTER], MM_DT, isOutput=False
    )
    w2_d = nc.declare_dram_parameter(
        "w2", [S, 128, JC * HIDDEN], MM_DT, isOutput=False
    )
    # Output layout: [128, OC, cap_total] fp16: out_d[p, oc, g] holds
    # out[token g][128*oc + p]. The oc-major layout keeps the PSUM->SBUF
    # casts contiguous (a token-interleaved layout made the DVE cast 4x
    # slower: strided 16B writes); the 3D store has tt*2-byte rows, which
    # is acceptable on the SWDGE trickle path.
    out_d = nc.declare_dram_parameter(
        "out", [128, OC, cap_total], f16, isOutput=True
    )

    big = ESZ == 4
    w13_bufs = 6 if big else 12
    w2_bufs = 2 if big else 3
    xt_bufs = 3 if big else 4

    with tile.TileContext(nc) as tc:
        with (
            tc.tile_pool(name="w13p", bufs=w13_bufs) as w13p,
            tc.tile_pool(name="w2p", bufs=w2_bufs) as w2p,
            tc.tile_pool(name="xtp", bufs=xt_bufs) as xtp,
            tc.tile_pool(name="hp", bufs=12) as hp,
            tc.tile_pool(name="sgp", bufs=6) as sgp,
            tc.tile_pool(name="outp", bufs=3) as outp,
            tc.tile_pool(name="ps1", bufs=4, space="PSUM") as ps1,
            tc.tile_pool(name="ps2", bufs=4, space="PSUM") as ps2,
        ):
            # All loads ride the sync HWDGE queue in issue order. (Spreading
            # loads across the sync+scalar HWDGE queues was measured to
            # HALVE both queues' per-engine rates - the 16 DMA engines are
            # shared - and the semaphore-pool recycling serialized issues;
            # first real matmul slipped from 12.6us to 18.6us.)
            # HAM warmup: PE sits at 1.2GHz until ~4us of sustained matmul
            # activity; run throwaway matmuls while the first DMAs fly.
            # Sized to bridge until the first weights land (~8.5us after
            # warmup start, gated by the slowest DMA engine): short MMs
            # until the clock flips, then long ones to cover the rest.
            # A gap >~3.4us here re-throttles HAM and halves the clock for
            # the first real matmuls - worse than over-warming by ~1us.
            warm_sb = sgp.tile([128, TT], MM_DT, tag="warm", name="warm_sb")
            nc.vector.memset(warm_sb[:], 0.0)
            warm_ps = ps1.tile([128, TT], f32, tag="pg", name="warm_ps",
                               padded_shape=[128, TT])
            for _w in range(WARMUP_SHORT):
                nc.tensor.matmul(
                    warm_ps[:, 0:128],
                    warm_sb[:, 0:128],
                    warm_sb[:, 0:128],
                    start=True,
                    stop=True,
                )
            for _w in range(WARMUP_LONG):
                nc.tensor.matmul(
                    warm_ps[:, 0:TT],
                    warm_sb[:, 0:128],
                    warm_sb[:, 0:TT],
                    start=True,
                    stop=True,
                )

            tix = 0
            for s in range(S):
                C = caps[s]
                off = int(offs[s])

                # First token tile's xt ahead of the weights: the queues
                # drain roughly in issue order and the first matmul of the
                # segment needs (xt, w13 pair 0). (Fine-grained k-chunk
                # splitting of segment 0 was tried and is a net loss: each
                # DMA's completion semaphore carries the slowest engine's
                # ~1-3us lag, so the k-paced loop starves and HAM
                # re-throttles.)
                tt0 = min(TT, C)
                xt_first = xtp.tile([128, KC * tt0], MM_DT, tag="xtt",
                                    name=f"xtt{tix}",
                                    padded_shape=[128, KC * TT])
                nc.sync.dma_start(out=xt_first[:], in_=xt_d[tix, :, 0 : KC * tt0])

                # Weights for this segment, in first-use (k) order.
                w13_t = []
                for kp in range(4):
                    w13t = w13p.tile([128, 2 * 2 * INTER], MM_DT, tag="w13t",
                                     name=f"w13t{s}_{kp}")
                    nc.sync.dma_start(out=w13t[:], in_=w13_d[s, kp])
                    w13_t.append(w13t)
                w2t = w2p.tile([128, JC * HIDDEN], MM_DT, tag="w2t", name=f"w2t{s}")
                nc.sync.dma_start(out=w2t[:], in_=w2_d[s])

                def w13_ap(k, m):
                    base = (k % 2) * 2 * INTER + 128 * m
                    return w13_t[k // 2][:, base : base + 128]

                def w2_ap(j, oc):
                    jw = min(128, INTER - 128 * j)
                    base = j * HIDDEN + 128 * oc
                    return w2t[0:jw, base : base + 128]

                for t0 in range(0, C, TT):
                    tt = min(TT, C - t0)
                    if t0 == 0:
                        xt_tile = xt_first
                    else:
                        xt_tile = xtp.tile([128, KC * tt], MM_DT, tag="xtt",
                                           name=f"xtt{tix}",
                                           padded_shape=[128, KC * TT])
                        nc.sync.dma_start(
                            out=xt_tile[:], in_=xt_d[tix, :, 0 : KC * tt]
                        )

                    def xt_ap(k):
                        return xt_tile[:, k * tt : (k + 1) * tt]

                    h_t = []
                    for j in range(JC):
                        jw = min(128, INTER - 128 * j)
                        ht = hp.tile([jw, tt], MM_DT, tag="ht", name=f"ht{tix}_{j}",
                                     padded_shape=[128, TT])
                        h_t.append(ht)

                    # GEMM1: k-outer within m-groups of 2 psum tiles.
                    for m_lo, m_hi in M_GROUPS:
                        pgs = {}
                        for m in range(m_lo, m_hi):
                            pgs[m] = ps1.tile([128, tt], f32, tag="pg",
                                              name=f"pg{m}",
                                              padded_shape=[128, TT])
                        for k in range(KC):
                            for m in range(m_lo, m_hi):
                                nc.tensor.matmul(
                                    pgs[m][:],
                                    w13_ap(k, m),
                                    xt_ap(k),
                                    start=(k == 0),
                                    stop=(k == KC - 1),
                                )
                        for m in range(m_lo, m_hi):
                            if m == MC - 1:
                                # tail chunk: [gate 64 | up 64] on partitions
                                sg = sgp.tile([64, tt], f32, tag="sg",
                                              name=f"sg{m}",
                                              padded_shape=[128, TT])
                                nc.scalar.activation(
                                    sg[:], pgs[m][0:64, :],
                                    mybir.ActivationFunctionType.Silu,
                                )
                                nc.vector.tensor_mul(
                                    h_t[JC - 1][0:64, :], sg[:],
                                    pgs[m][64:128, :],
                                )
                            elif m % 2 == 1:
                                sg = sgp.tile([128, tt], f32, tag="sg",
                                              name=f"sg{m}",
                                              padded_shape=[128, TT])
                                nc.scalar.activation(
                                    sg[:], pgs[m - 1][:],
                                    mybir.ActivationFunctionType.Silu,
                                )
                                nc.vector.tensor_mul(
                                    h_t[m // 2][:], sg[:], pgs[m][:]
                                )

                    # GEMM2: w2 stationary, h moving; output [oc*128, tokens]
                    # in PSUM, cast (contiguously) to fp16 into one out tile
                    # and stored with a 3D-AP DMA per token tile.
                    ob = outp.tile([128, OC * tt], f16, tag="ob", name=f"ob{tix}",
                                   padded_shape=[128, OC * TT])
                    for half in range(2):
                        pos = {}
                        for oc in range(4 * half, 4 * half + 4):
                            pos[oc] = ps2.tile([128, tt], f32, tag="po",
                                               name=f"po{oc}",
                                               padded_shape=[128, TT])
                        for j in range(JC):
                            for oc in range(4 * half, 4 * half + 4):
                                nc.tensor.matmul(
                                    pos[oc][:],
                                    w2_ap(j, oc),
                                    h_t[j][:],
                                    start=(j == 0),
                                    stop=(j == JC - 1),
                                )
                        for oc in range(4 * half, 4 * half + 4):
                            nc.vector.tensor_copy(
                                ob[:, oc * tt : (oc + 1) * tt], pos[oc][:]
                            )

                    # Stores ride the gpsimd (SWDGE) queue so they never
                    # block later loads on the sync sequencer; the last
                    # segment's store takes the HWDGE path. (Do NOT touch
                    # the scalar HWDGE queue: merely allocating the second
                    # HWDGE ring slowed the head's load stream by ~3us.)
                    src = ob[:].rearrange("p (o t) -> p o t", o=OC)
                    store_eng = nc.sync if s == S - 1 else nc.gpsimd
                    store_eng.dma_start(
                        out=out_d[:, :, off + t0 : off + t0 + tt],
                        in_=src,
                    )
                    tix += 1

    nc.compile()
    return nc


_BUILD_CACHE = {}


def _get_program(S, caps, cap_total):
    key = (S, tuple(caps), str(MM_DT))
    if key not in _BUILD_CACHE:
        _BUILD_CACHE[key] = _build(S, caps, cap_total)
    return _BUILD_CACHE[key]


def _pack_inputs(x, assign, caps, offs, cap_total, w13_perm):
    """Build per-core input dicts matching the device layouts."""
    tiles = _tiles_of(caps)
    NT = len(tiles)
    S = len(caps)
    in_maps = []
    for c in range(NCORES):
        xt_c = np.zeros((HIDDEN, cap_total), dtype=NP_DT)
        w13_c = np.zeros((S, 4, 128, 2 * 2 * INTER), dtype=NP_DT)
        w2_c = np.zeros((S, 128, JC * HIDDEN), dtype=NP_DT)
        for s, (e, a, n) in enumerate(assign[c]):
            if e is None or n <= 0:
                continue
            o = int(offs[s])
            xt_c[:, o : o + n] = _to_np_dt(x[a : a + n, :]).T
            w13_c[s] = (
                w13_perm["w13"][e]
                .reshape(4, 2, 128, 2 * INTER)
                .transpose(0, 2, 1, 3)
                .reshape(4, 128, 2 * 2 * INTER)
            )
            w2_c[s] = w13_perm["w2"][e]
        xt_pack = np.zeros((NT, 128, KC * TT), dtype=NP_DT)
        for tix, (s, t0, tt) in enumerate(tiles):
            o = int(offs[s])
            blk = xt_c[:, o + t0 : o + t0 + tt]
            xt_pack[tix, :, 0 : KC * tt] = (
                blk.reshape(KC, 128, tt).transpose(1, 0, 2).reshape(128, KC * tt)
            )
        in_maps.append({"xt": xt_pack, "w13": w13_c, "w2": w2_c})
    return in_maps


def _prep_weights(w1w3, w2):
    w13_perm = _to_np_dt(w1w3[:, :, _PERM])
    w2p_all = np.zeros((N_EXPERTS, JC * 128, HIDDEN), dtype=NP_DT)
    w2p_all[:, :INTER] = _to_np_dt(w2)
    w2_pack = (
        w2p_all.reshape(N_EXPERTS, JC, 128, HIDDEN)
        .transpose(0, 2, 1, 3)
        .reshape(N_EXPERTS, 128, JC * HIDDEN)
    )
    return {"w13": w13_perm, "w2": w2_pack}


def _run(x, tokens_per_expert, w1w3, w2, trace=False):
    x = np.ascontiguousarray(np.asarray(x, dtype=np.float32))
    counts = np.asarray(tokens_per_expert, dtype=np.int64).copy()
    w1w3 = np.asarray(w1w3, dtype=np.float32)
    w2 = np.asarray(w2, dtype=np.float32)

    T = x.shape[0]
    # Clip group sizes like ragged_dot: groups are consecutive; anything
    # beyond T is out of range.
    counts = np.maximum(counts, 0)
    cum = np.cumsum(counts)
    over = cum > T
    if over.any():
        first = int(np.argmax(over))
        prev = int(cum[first - 1]) if first > 0 else 0
        counts[first] = T - prev
        counts[first + 1 :] = 0

    assign, caps, offs, cap_total = _plan(counts)
    S = len(caps)
    nc = _get_program(S, caps, cap_total)

    packed_w = _prep_weights(w1w3, w2)
    in_maps = _pack_inputs(x, assign, caps, offs, cap_total, packed_w)

    extra = {}
    if trace:
        import os

        os.makedirs("/tmp/moe_prof", exist_ok=True)
        for f in os.listdir("/tmp/moe_prof"):
            os.unlink(os.path.join("/tmp/moe_prof", f))
        extra["tmpdir"] = "/tmp/moe_prof"
    res = run_bass_kernel_spmd(nc, in_maps, list(range(NCORES)), trace=trace, **extra)

    out_full = np.zeros((T, HIDDEN), dtype=np.float32)
    for c in range(NCORES):
        oc = res.results[c]["out"]  # [128, OC, cap_total] fp16
        arr = oc.transpose(1, 0, 2).reshape(HIDDEN, cap_total)
        for s, (e, a, n) in enumerate(assign[c]):
            if e is None or n <= 0:
                continue
            o = int(offs[s])
            out_full[a : a + n, :] = arr[:, o : o + n].T.astype(np.float32)
    return out_full, res


def kernel(x, tokens_per_expert, w1w3, w2, decoding=False, **_ignored):
    out, _ = _run(x, tokens_per_expert, w1w3, w2, trace=False)
    return out
